# revision 119
# baseline (speedup 1.0000x reference)
"""AnomalyTransformer forward pass on 8 Trainium2 NeuronCores.

Data-parallel over batch: each core processes 32 of the 256 batch elements
through the full 3-layer transformer.

Precision strategy: the residual stream h and the q/k-path projection
weights run in float32r (TF32-like TensorEngine mode, full throughput at
moving-dim >= 256); q/k chunk tiles, the value path and attention output
run in bf16. Softmax logits reach +-38 in layer 3, so an all-bf16 kernel
amplifies rounding to ~1.5e-2 relative error; this mix lands at ~6.4e-3
(measured) against the f32 reference, with the Tile cost-model timeline at
~428 us per core (PE busy ~394 us, ~92% occupancy).

Layout strategy: the residual stream h is feature-major ([D, tokens], D
split over 4 partition-tiles of 128). Attention uses the scoresT
orientation (scoresT = khT.T @ qhT -> [l_k, l_q]): exp runs on ACT into
bf16, per-head sum(exp) comes from tiny ones-vector matmuls, and all 8
heads' unnormalized outputs share one PSUM bank (64 cols each); one DVE
reciprocal + one broadcast multiply normalizes. A per-batch PE transpose
returns the output to feature-major for the Wo projection, with residual
adds on DVE reading the Wo/W2 PSUM directly.

Scheduling: everything is software-pipelined against the PE program order —
next chunk's q/k/v projections are emitted in 12 pieces through the current
chunk's batch loop (so exp latency and PSUM-ring recycles hide under PE
work), transposes run 4 batches behind the normalize, the FFN processes
chunk pairs through one 128-row gelu (second chunk's W1/W2 use zero-padded
weight halves since nonzero tile_position is ISA-illegal for 4-byte
dtypes), and the last layer's FFN+output projection interleave through the
freed score banks. PSUM banks: pp 3 / scores 3 / ffn 1 / vps 1. Engine
routing of PSUM->SBUF copies is load-balanced between ACT and DVE (the
Pool engine cannot touch PSUM). Input xcat and the first chunks' weights
stream via split DMAs on two HWDGE queues to shorten the startup ramp.

The sigma/prior branch of the reference is dead code (never feeds the
output) and is skipped. Biases in the reference are all zeros and are
skipped.
"""

import sys
import os
for _p in ("/opt/trn_rl_repo", "/root/.axon_site/_ro/trn_rl_repo"):
    if os.path.isdir(_p) and _p not in sys.path:
        sys.path.insert(0, _p)

import math
import numpy as np
import ml_dtypes

import concourse.bass as bass
import concourse.tile as tile
from concourse import mybir
from concourse.bass_utils import run_bass_kernel_spmd
from contextlib import ExitStack

BF16 = mybir.dt.bfloat16
F32 = mybir.dt.float32
F8 = mybir.dt.float8e4
F32R = mybir.dt.bfloat16 if os.environ.get("ANOM_RDT", "f32r") == "bf16" else mybir.dt.float32r
AF = mybir.ActivationFunctionType
OP = mybir.AluOpType
MPM = mybir.MatmulPerfMode
WODR = os.environ.get("ANOM_WODR", "0") == "1"   # Wo matmul in fp8 DoubleRow
WODT = F8 if WODR else BF16
OUM = os.environ.get("ANOM_OUM", "1") == "1"     # single-bank ou + sums matmuls
FFNPAIR = os.environ.get("ANOM_FFNPAIR", "1") == "1"  # 2 chunks per gelu
XBF = os.environ.get("ANOM_XBF", "0") == "1"    # bf16 xcat/wemb (halves input DMA)

# model dims
B, L, C, D, H, NL, DFF = 256, 100, 55, 512, 8, 3, 64
DK = D // H                      # 64
NCORES = 8
BL = B // NCORES                 # 32 batches per core
TOK = BL * L                     # 3200 tokens per core
TCH = 400                        # token chunk (4 batches)
NT = TOK // TCH                  # 8 chunks
CB = TCH // L                    # 4 batches per chunk
KT = D // 128                    # 4 contraction tiles
C3 = 3 * C                       # 165 unfolded conv rows


_NOSTRUCT = ("InstDrain", "InstNoOp", "InstEventSemaphore", "InstHalt")


def _legalize_waits(nc, maxw=1):
    """This container's walrus caps sync-waits at 1 per instruction; move
    extra waits onto preceding same-engine NOPs (one wait each)."""
    cnt = [0]
    for f in nc.m.functions:
        for blk in f.blocks:
            newlist = []
            changed = False
            for ins in blk.instructions:
                si = getattr(ins, "sync_info", None)
                lim = maxw
                if si is not None and si.on_wait and len(si.on_wait) > lim:
                    waits = list(si.on_wait)
                    extra, keep = waits[:-lim], waits[-lim:]
                    for i in range(0, len(extra), 1):
                        cnt[0] += 1
                        nop = mybir.InstNoOp(
                            name=f"I-ws-{cnt[0]}", ins=[], outs=[], engine=ins.engine
                        )
                        nop.sync_info = mybir.SyncInfo(
                            on_wait=extra[i:i + 1], on_update=[]
                        )
                        newlist.append(nop)
                    ins.sync_info = mybir.SyncInfo(
                        on_wait=keep, on_update=list(si.on_update)
                    )
                    changed = True
                newlist.append(ins)
            if changed:
                blk.instructions = newlist
    return nc


def _offset_ap(ap, extra_offset, dims):
    """AP at ap.offset + extra_offset (elements) with free dims `dims`
    ([[step, count], ...]), keeping ap's partition dim."""
    return bass.AP(tensor=ap.tensor, offset=ap.offset + extra_offset,
                   ap=[list(ap.ap[0])] + [list(d) for d in dims])


def build_nc():
    RES_DVE = os.environ.get("ANOM_RESDVE", "1") == "1"
    QK_BF = os.environ.get("ANOM_QKBF", "1") == "1"
    QKDT = BF16 if QK_BF else F32R
    QK_ACT = os.environ.get("ANOM_QKACT", "1") == "1"
    OT_DVE = os.environ.get("ANOM_OTDVE", "1") == "1"
    V_ACT = os.environ.get("ANOM_VACT", "1") == "1"
    nc = bass.Bass()

    # engine routing: 'a'=ACT (copy only), 'd'=DVE, 'p'=Pool/GpSimd
    def _copy_eng(ch):
        return {"a": nc.scalar.copy, "d": nc.vector.tensor_copy,
                "p": nc.gpsimd.tensor_copy}[ch]

    def _tt_eng(ch):
        return {"d": nc.vector, "p": nc.gpsimd}[ch]

    QKENG = os.environ.get("ANOM_QKENG", "dddd")
    KKENG = os.environ.get("ANOM_KKENG", "adaa")
    VENG = os.environ.get("ANOM_VENG", "a" if V_ACT else "d")
    OENG = os.environ.get("ANOM_OENG", "dd")
    OTENG = os.environ.get("ANOM_OTENG", "d" if OT_DVE else "a")
    R1ENG = os.environ.get("ANOM_R1ENG", "dddd")
    R2ENG = os.environ.get("ANOM_R2ENG", "dddd")
    RCENG = os.environ.get("ANOM_RCENG", "dd")
    VPOOL = os.environ.get("ANOM_VPOOL", "tpp")   # tpp|oup|pp pool for vps
    TPPIPE = int(os.environ.get("ANOM_TPPIPE", "4"))  # transpose delay (batches)
    WOSPLIT = os.environ.get("ANOM_WOSPLIT", "1") == "1"  # Wo per-batch cols
    FFNI = os.environ.get("ANOM_FFNI", "0") == "1"  # interleave ffn into attn
    FFNPIPE = os.environ.get("ANOM_FFNPIPE", "0") == "1"  # pipeline W1 vs W2
    VINT = os.environ.get("ANOM_VINT", "0") == "1"  # interleave V into batches
    TPPOOL = os.environ.get("ANOM_TPPOOL", "p")  # p=pp ring, s=score ring

    # ---- DRAM parameters (host-prepped) ----
    XDT = BF16 if XBF else F32R
    xcat0 = nc.declare_dram_parameter("xcat0", [128, TOK], XDT, isOutput=False)
    xcat1 = nc.declare_dram_parameter("xcat1", [C3 - 128, TOK], XDT, isOutput=False)
    wemb0 = nc.declare_dram_parameter("wemb0", [128, D], XDT, isOutput=False)
    wemb1 = nc.declare_dram_parameter("wemb1", [C3 - 128, D], XDT, isOutput=False)
    pe_d = nc.declare_dram_parameter("pe", [128, KT, L], F32, isOutput=False)
    wqkv_d = nc.declare_dram_parameter("wqkv", [128, NL, 3, KT, D], F32R,
                                       isOutput=False)
    # fp8 weights ship as uint8 bytes (axon/jax can't transfer fp8 dtypes)
    wo_d = nc.declare_dram_parameter("wo", [128, NL, KT, D],
                                     mybir.dt.uint8 if WODR else BF16,
                                     isOutput=False)
    w1_d = nc.declare_dram_parameter("w1", [128, NL, KT, DFF], F32R, isOutput=False)
    w2_d = nc.declare_dram_parameter("w2", [DFF, NL, KT, 128], F32R, isOutput=False)
    wout_d = nc.declare_dram_parameter("wout", [128, KT, C], F32R, isOutput=False)
    identb_d = nc.declare_dram_parameter("identb", [128, 128], BF16, isOutput=False)
    identr_d = nc.declare_dram_parameter("identr", [128, 128], F32R, isOutput=False)
    out_d = nc.declare_dram_parameter("out", [C, TOK], F32, isOutput=True)

    with tile.TileContext(nc) as tc, ExitStack() as stk:
        tc.race_detector_enabled = False
        singles = stk.enter_context(tc.tile_pool(name="singles", bufs=1))
        wp = stk.enter_context(tc.tile_pool(name="wp", bufs=int(os.environ.get("ANOM_WPB", "2"))))
        xp = stk.enter_context(tc.tile_pool(name="xp", bufs=int(os.environ.get("ANOM_XPB", "4"))))
        qp = stk.enter_context(tc.tile_pool(name="qp", bufs=int(os.environ.get("ANOM_QB", "2"))))
        kp = stk.enter_context(tc.tile_pool(name="kp", bufs=int(os.environ.get("ANOM_QB", "2"))))
        vp = stk.enter_context(tc.tile_pool(name="vp", bufs=int(os.environ.get("ANOM_VB", "2"))))
        expp = stk.enter_context(tc.tile_pool(name="expp", bufs=int(os.environ.get("ANOM_EXB", "3"))))
        op_ = stk.enter_context(tc.tile_pool(name="op", bufs=int(os.environ.get("ANOM_OB", "3"))))
        rp = stk.enter_context(tc.tile_pool(name="rp", bufs=3))
        otp = stk.enter_context(tc.tile_pool(name="otp", bufs=int(os.environ.get("ANOM_OTB", "2"))))
        yp = stk.enter_context(tc.tile_pool(name="yp", bufs=2))
        outp = stk.enter_context(tc.tile_pool(name="outp", bufs=2))
        # psum pools (8 banks total)
        pp = stk.enter_context(tc.tile_pool(name="pp", bufs=int(os.environ.get("ANOM_PPB", "3")), space="PSUM"))
        scp = stk.enter_context(tc.tile_pool(name="scp", bufs=int(os.environ.get("ANOM_SCB", "3")), space="PSUM"))
        oup = stk.enter_context(tc.tile_pool(name="oup", bufs=int(os.environ.get("ANOM_OUB", "1")), space="PSUM"))
        tpp = stk.enter_context(tc.tile_pool(name="tpp", bufs=int(os.environ.get("ANOM_TPB", "1")), space="PSUM"))

        # ---- persistent SBUF ----
        wemb0_sb = singles.tile([128, D], XDT)
        wemb1_sb = singles.tile([C3 - 128, D], XDT)
        pe_sb = singles.tile([128, KT, L], F32)
        wout_sb = singles.tile([128, KT, C], F32R)
        ident_b = singles.tile([128, 128], BF16)
        ident_r = singles.tile([128, 128], F32R)
        ones_sb = singles.tile([128, 1], BF16)
        nc.vector.memset(ones_sb[:], 1.0)
        h_sb = [singles.tile([128, TOK], F32R, name=f"h{k}") for k in range(KT)]

        # ---- token embedding: circular conv as matmul, + positional emb ----
        # xcat0 DMAs go first on the SP HWDGE queue (its ~625ns per-DMA
        # dispatch serializes startup); the shared singles go via the ACT
        # engine's separate HWDGE queue. xcat1 (37 rows) loads once.
        xc1_sb = singles.tile([C3 - 128, TOK], XDT)
        xc0s = []
        for t2 in range(0, NT, 2):
            xc0 = xp.tile([128, 2 * TCH], XDT, tag="xc0")
            nc.sync.dma_start(out=xc0[:],
                              in_=xcat0[:, t2 * TCH:(t2 + 2) * TCH])
            xc0s.append(xc0)
        nc.scalar.dma_start(out=wemb0_sb[:], in_=wemb0[:])
        # xcat1 split so chunk 0/1's slice lands before the first wemb1
        # matmul instead of behind the whole 3200-column transfer
        nc.scalar.dma_start(out=xc1_sb[:, :2 * TCH], in_=xcat1[:, :2 * TCH])
        nc.scalar.dma_start(out=wemb1_sb[:], in_=wemb1[:])
        nc.scalar.dma_start(out=pe_sb[:], in_=pe_d[:])
        nc.scalar.dma_start(out=xc1_sb[:, 2 * TCH:], in_=xcat1[:, 2 * TCH:])
        for dst, src in ((wout_sb, wout_d),
                         (ident_b, identb_d), (ident_r, identr_d)):
            nc.scalar.dma_start(out=dst[:], in_=src[:])
        def emit_emb(t):
            # the score banks are idle until the first attention chunk, so
            # the embedding psum rotates through them, keeping the pp ring
            # free for the interleaved layer-0 q/k/v prologue
            xc0 = xc0s[t // 2]
            tt = t % 2
            tsl = slice(t * TCH, (t + 1) * TCH)
            for m in range(KT):
                ps = scp.tile([128, 512], F32, tag="sc", name="embps")
                nc.tensor.matmul(ps[:, :TCH],
                                 wemb0_sb[:, m * 128:(m + 1) * 128],
                                 xc0[:, tt * TCH:(tt + 1) * TCH],
                                 start=True, stop=False)
                nc.tensor.matmul(ps[:, :TCH],
                                 wemb1_sb[:, m * 128:(m + 1) * 128],
                                 xc1_sb[:, tsl], start=False, stop=True)
                pe_b = _offset_ap(pe_sb[:, m, :], 0, [[0, CB], [1, L]])
                nc.vector.tensor_tensor(
                    h_sb[m][:, tsl].rearrange("p (b x) -> p b x", x=L),
                    ps[:, :TCH].rearrange("p (b x) -> p b x", x=L),
                    pe_b, op=OP.add)

        EMBI = os.environ.get("ANOM_EMBI", "0") == "1"
        PROI = os.environ.get("ANOM_PROI", "1") == "1"
        emb_todo = []
        if PROI:
            # first two chunks now; the rest interleave with the layer-0
            # q/k/v prologue so its PE work hides under the DVE-paced
            # embedding adds
            emit_emb(0)
            emit_emb(1)
            emb_todo = list(range(2, NT))
        else:
            for t in range(2 if EMBI else NT):
                emit_emb(t)

        # ---- transformer layers ----
        n_layer_passes = int(os.environ.get("ANOM_LAYERS", str(NL)))
        LPIPE = os.environ.get("ANOM_LPIPE", "0") == "1"

        def load_weights(l):
            wqkv_t = wp.tile([128, 3, KT, D], F32R, tag="wqkv")
            # split per matrix so layer 0's Q third lands as soon as the
            # embedding's first chunk is ready instead of behind the full
            # 3 MB transfer
            for i in range(3):
                nc.sync.dma_start(out=wqkv_t[:, i], in_=wqkv_d[:, l, i])
            wo_t = wp.tile([128, KT, D],
                           mybir.dt.uint8 if WODR else BF16, tag="wo")
            nc.sync.dma_start(out=wo_t[:], in_=wo_d[:, l])
            w2_t = wp.tile([DFF, KT, 128], F32R, tag="w2")
            nc.sync.dma_start(out=w2_t[:], in_=w2_d[:, l])
            if FFNPAIR:
                # FFN chunk pairing without tile_position (the ISA rejects
                # nonzero tile_position for 4-byte dtypes): the second
                # chunk's W1 lhsT is zero-padded in free cols 0-63 so its
                # outputs land on partitions 64-127; its W2 lhsT is
                # zero-padded in partitions 0-63 so a full-128 contraction
                # ignores the first chunk's gelu rows.
                w1_t = wp.tile([128, KT, 2 * DFF], F32R, tag="w1")
                nc.gpsimd.memset(w1_t[:, :, :DFF].bitcast(F32), 0.0)
                nc.sync.dma_start(out=w1_t[:, :, DFF:], in_=w1_d[:, l])
                w2b_t = wp.tile([2 * DFF, KT, 128], F32R, tag="w2b")
                nc.gpsimd.memset(w2b_t[:DFF].bitcast(F32), 0.0)
                nc.sync.dma_start(out=w2b_t[DFF:], in_=w2_d[:, l])
            else:
                w1_t = wp.tile([128, KT, DFF], F32R, tag="w1")
                nc.sync.dma_start(out=w1_t[:], in_=w1_d[:, l])
                w2b_t = None
            return wqkv_t, wo_t, w1_t, w2_t, w2b_t

        wt_cur = load_weights(0)
        carry = None  # next layer's chunk-0 qkv, emitted during this FFN
        for lp_i in range(n_layer_passes):
            l = lp_i % NL
            if wt_cur is None:
                wt_cur = load_weights(l)
            wqkv_l, wo_l, w1_l, w2_l, w2b_l = wt_cur
            wt_next = (load_weights((lp_i + 1) % NL)
                       if LPIPE and lp_i + 1 < n_layer_passes else None)

            FFNP = os.environ.get("ANOM_FFNP", "1") == "1"
            TPSHARE = os.environ.get("ANOM_TPSHARE", "1") == "1"
            vpool = {"tpp": tpp, "oup": oup, "pp": pp}[VPOOL]

            def emit_v(g, bi, v_t, wv=None):
                wv = wqkv_l if wv is None else wv
                b = g * CB + bi
                bsl = slice(b * L, (b + 1) * L)
                ps = vpool.tile([128, 512], F32,
                                tag="pp" if VPOOL == "pp" else "vps", name="vps")
                for k in range(KT):
                    nc.tensor.matmul(ps[:L, :], h_sb[k][:, bsl],
                                     wv[:, 2, k, :],
                                     start=(k == 0), stop=(k == KT - 1))
                if OUM:
                    _copy_eng(VENG)(v_t[:L, bi, :], ps[:L, :])
                else:
                    _copy_eng(VENG)(
                        v_t[:L, bi, :].rearrange(
                            "p (h x) -> p h x", x=65)[:, :, :64],
                        ps[:L, :].rearrange("p (h x) -> p h x", x=64))

            def emit_tp(bi, o_t, ot_all):
                # transpose o back to feature-major: all 4 m-chunks into
                # one psum tile, then one strided copy into ot_all
                if TPPOOL == "s":
                    tp = scp.tile([128, 1024], BF16, tag="sc", name="tp")
                elif TPSHARE:
                    tp = pp.tile([128, 1024], BF16, tag="pp", name="tp")
                else:
                    tp = tpp.tile([128, 1024], BF16, tag="tp")
                for m in range(KT):
                    nc.tensor.transpose(tp[:, m * L:(m + 1) * L],
                                        o_t[:L, m * 128:(m + 1) * 128],
                                        ident_b[:L, :L])
                _copy_eng(OTENG)(
                    _offset_ap(ot_all[:, :, :], bi * L, [[TCH, KT], [1, L]]),
                    tp[:, :KT * L].rearrange("p (m x) -> p m x", x=L))

            def emit_ffn1(g):
                gsl = slice(g * TCH, (g + 1) * TCH)
                if FFNPIPE:
                    ps1 = tpp.tile([128, 512], F32, tag="ffn1", name="ps1")
                else:
                    ps1 = (oup if FFNP else pp).tile([128, 512], F32,
                                                     tag="ffn" if FFNP else "pp",
                                                     name="ps1")
                for k in range(KT):
                    w1r = w1_l[:, k, DFF:] if FFNPAIR else w1_l[:, k, :]
                    nc.tensor.matmul(ps1[:DFF, :TCH], w1r,
                                     h_sb[k][:, gsl],
                                     start=(k == 0), stop=(k == KT - 1))
                y_t = yp.tile([DFF, TCH], F32R, tag="y")
                nc.scalar.activation(y_t[:, :], ps1[:DFF, :TCH], AF.Gelu)
                return y_t

            def emit_ffn2(g, y_t):
                gsl = slice(g * TCH, (g + 1) * TCH)
                for m in range(KT):
                    ps2 = (oup if FFNP else pp).tile([128, 512], F32,
                                                     tag="ffn" if FFNP else "pp",
                                                     name="ps2")
                    if RES_DVE:
                        nc.tensor.matmul(ps2[:, :TCH], w2_l[:, m, :], y_t[:, :],
                                         start=True, stop=True)
                        _tt_eng(R2ENG[m]).tensor_tensor(
                            h_sb[m][:, gsl], ps2[:, :TCH],
                            h_sb[m][:, gsl], op=OP.add)
                    else:
                        nc.tensor.matmul(ps2[:, :TCH], w2_l[:, m, :], y_t[:, :],
                                         start=True, stop=False)
                        nc.tensor.matmul(ps2[:, :TCH], ident_r[:], h_sb[m][:, gsl],
                                         start=False, stop=True)
                        nc.scalar.copy(h_sb[m][:, gsl], ps2[:, :TCH])

            def emit_ffn(g):
                emit_ffn2(g, emit_ffn1(g))

            def emit_ffn_pair(ga, gb, fill=None, after_half=None):
                # two chunks share one ps1 bank (partitions 0-63 / 64-127)
                # and one gelu. Nonzero tile_position is ISA-illegal for
                # 4-byte dtypes, so chunk b goes first with a W1 lhsT whose
                # free cols 0-63 are zero (outputs land on partitions
                # 64-127, rows 0-63 init to 0); chunk a then accumulates
                # into rows 0-63. Chunk b's W2 contracts all 128 gelu rows
                # against w2b_l, whose partitions 0-63 are zero.
                gsl_a = slice(ga * TCH, (ga + 1) * TCH)
                gsl_b = slice(gb * TCH, (gb + 1) * TCH)
                ps1 = (oup if FFNP else pp).tile([128, 512], F32,
                                                 tag="ffn" if FFNP else "pp",
                                                 name="ps1")
                for k in range(KT):
                    nc.tensor.matmul(ps1[:, :TCH],
                                     w1_l[:, k, :], h_sb[k][:, gsl_b],
                                     start=(k == 0), stop=False)
                for k in range(KT):
                    nc.tensor.matmul(ps1[:DFF, :TCH],
                                     w1_l[:, k, DFF:], h_sb[k][:, gsl_a],
                                     start=False, stop=(k == KT - 1))
                y2 = yp.tile([128, TCH], F32R, tag="y")
                nc.scalar.activation(y2[:, :], ps1[:, :TCH], AF.Gelu)
                for half, gsl in ((0, gsl_a), (1, gsl_b)):
                    for m in range(KT):
                        if fill is not None:
                            fill()
                        if tail_ffn:
                            # attention is over: the score banks are free,
                            # so the last layer's W2 psum rotates through
                            # them instead of the single ffn bank
                            ps2 = scp.tile([128, 512], F32, tag="sc",
                                           name="ps2")
                        else:
                            ps2 = (oup if FFNP else pp).tile(
                                [128, 512], F32,
                                tag="ffn" if FFNP else "pp", name="ps2")
                        if half == 0:
                            w2r, y2r = w2_l[:, m, :], y2[:DFF, :]
                        else:
                            w2r, y2r = w2b_l[:, m, :], y2[:, :]
                        r2 = TAILR2[m] if tail_ffn else R2ENG[m]
                        if r2 == "a":
                            # residual via identity accumulate + ACT copy:
                            # drains the psum ring in parallel with the
                            # DVE adds of the other m-tiles
                            nc.tensor.matmul(ps2[:, :TCH], w2r, y2r,
                                             start=True, stop=False)
                            nc.tensor.matmul(ps2[:, :TCH], ident_r[:],
                                             h_sb[m][:, gsl],
                                             start=False, stop=True)
                            nc.scalar.copy(h_sb[m][:, gsl], ps2[:, :TCH])
                        else:
                            nc.tensor.matmul(ps2[:, :TCH], w2r, y2r,
                                             start=True, stop=True)
                            _tt_eng(r2).tensor_tensor(
                                h_sb[m][:, gsl], ps2[:, :TCH],
                                h_sb[m][:, gsl], op=OP.add)
                    if after_half is not None:
                        after_half(ga if half == 0 else gb)

            QKVPIPE = os.environ.get("ANOM_QKVPIPE", "1") == "1"

            def emit_qk(g, qk, m, qc_m, wq=None):
                wq = wqkv_l if wq is None else wq
                gsl = slice(g * TCH, (g + 1) * TCH)
                msl = slice(m * 128, (m + 1) * 128)
                ps = pp.tile([128, 512], F32, tag="pp")
                for k in range(KT):
                    nc.tensor.matmul(ps[:, :TCH], wq[:, qk, k, msl],
                                     h_sb[k][:, gsl],
                                     start=(k == 0), stop=(k == KT - 1))
                _copy_eng((QKENG if qk == 0 else KKENG)[m])(qc_m[:],
                                                            ps[:, :TCH])

            def alloc_qkv(g, wq=None):
                qc = [qp.tile([128, TCH], QKDT, name=f"qc{m}", tag=f"qc{m}")
                      for m in range(KT)]
                kc = [kp.tile([128, TCH], QKDT, name=f"kc{m}", tag=f"kc{m}")
                      for m in range(KT)]
                if OUM:
                    v_t = vp.tile([128, CB, 512], BF16, tag="v")
                else:
                    v_t = vp.tile([128, CB, 8 * 65], BF16, tag="v")
                    nc.vector.memset(
                        v_t[:L, :, :].rearrange(
                            "p b (h x) -> p b h x", x=65)[:, :, :, 64:65], 1.0)
                # 12 emission units: 4 q, 4 k, 4 v — either run now or
                # spread through the previous chunk's batch loop
                units = ([lambda m=m: emit_qk(g, 0, m, qc[m], wq) for m in range(KT)]
                         + [lambda m=m: emit_qk(g, 1, m, kc[m], wq) for m in range(KT)]
                         + [lambda bi=bi: emit_v(g, bi, v_t, wq) for bi in range(CB)])
                return qc, kc, v_t, units

            if carry is None:
                cur = alloc_qkv(0)
                units0 = list(cur[3])
                while emb_todo:
                    emit_emb(emb_todo.pop(0))
                    for _ in range(2):
                        if units0:
                            units0.pop(0)()
                while units0:
                    units0.pop(0)()
            else:
                cur = carry
            for g in range(NT):
                gsl = slice(g * TCH, (g + 1) * TCH)
                qc, kc, v_t, _ = cur
                nxt = None
                if QKVPIPE and g + 1 < NT:
                    nxt = alloc_qkv(g + 1)
                    nxt_units = list(nxt[3])
                else:
                    nxt_units = []

                # interleave previous chunk's FFN into this chunk's attention
                if FFNI and g > 0:
                    emit_ffn(g - 1)
                # layer 0: interleave the embedding of chunk g+2 (PE filler
                # that hides the xcat DMA latency under attention compute)
                if EMBI and lp_i == 0 and g + 2 < NT:
                    emit_emb(g + 2)

                ot_all = otp.tile([128, KT, TCH], WODT, tag="ot")
                pend = []
                for bi in range(CB):
                    csl = slice(bi * L, (bi + 1) * L)
                    # scoresT for 8 heads: even heads -> scA, odd -> scB
                    # (different PE row groups must write different PSUM banks)
                    scA = scp.tile([128, 512], F32, tag="sc")
                    scB = scp.tile([128, 512], F32, tag="sc")
                    # evens first so the scA exp can start after 4 matmuls
                    for hh in (0, 2, 4, 6, 1, 3, 5, 7):
                        kt_i, base = divmod(hh * DK, 128)
                        sc = scA if hh % 2 == 0 else scB
                        col = (hh // 2) * 128
                        nc.tensor.matmul(sc[:L, col:col + L],
                                         kc[kt_i][base:base + DK, csl],
                                         qc[kt_i][base:base + DK, csl],
                                         start=True, stop=True)
                    if VINT and bi + 1 < CB:
                        # next batch's V matmuls fill PE while exp runs on ACT
                        emit_v(g, bi + 1, v_t)
                    exp_t = expp.tile([128, 8 * L], BF16, tag="exp")
                    # exp; head hh lands at exp_t cols hh*L
                    nc.scalar.activation(
                        exp_t[:L, :].rearrange("p (h x) -> p h x", x=2 * L)[:, :, :L],
                        scA[:L, :].rearrange("p (h x) -> p h x", x=128)[:, :, :L],
                        AF.Exp)
                    nc.scalar.activation(
                        _offset_ap(exp_t[:L, :], L, [[2 * L, 4], [1, L]]),
                        scB[:L, :].rearrange("p (h x) -> p h x", x=128)[:, :, :L],
                        AF.Exp)
                    # next chunk's q/k/v pieces: PE filler while exp drains
                    # on ACT; their ACT copies queue behind this batch's exp
                    for _ in range(3):
                        if nxt_units:
                            nxt_units.pop(0)()
                    if OUM:
                        # sums first so DVE reciprocal overlaps the ou
                        # matmuls; all 8 head sums + 8 head outputs are
                        # single-write regions (bank pending-zero safe)
                        sums = ((scp.tile([128, 512], F32, tag="sc",
                                          name="sums"))
                                if os.environ.get("ANOM_SUMSP", "0") == "1"
                                else pp.tile([128, 512], F32, tag="pp",
                                             name="sums"))
                        for hh in range(8):
                            nc.tensor.matmul(sums[:L, hh:hh + 1],
                                             exp_t[:L, hh * L:(hh + 1) * L],
                                             ones_sb[:L, :1],
                                             start=True, stop=True)
                        ou1 = pp.tile([128, 512], F32, tag="pp", name="ou1")
                        for hh in range(8):
                            nc.tensor.matmul(ou1[:L, hh * 64:(hh + 1) * 64],
                                             exp_t[:L, hh * L:(hh + 1) * L],
                                             v_t[:L, bi, hh * 64:(hh + 1) * 64],
                                             start=True, stop=True)
                    else:
                        if os.environ.get("ANOM_OUSHARE", "1") == "1":
                            ouA = pp.tile([128, 512], F32, tag="pp", name="ouA")
                            ouB = pp.tile([128, 512], F32, tag="pp", name="ouB")
                        else:
                            ouA = oup.tile([128, 512], F32, tag="ou")
                            ouB = oup.tile([128, 512], F32, tag="ou")
                        # evens first: they only need the scA exp, and run
                        # while the scB exp is still on ACT
                        for hh in (0, 2, 4, 6, 1, 3, 5, 7):
                            ou = ouA if hh % 2 == 0 else ouB
                            col = (hh // 2) * 128
                            nc.tensor.matmul(ou[:L, col:col + 65],
                                             exp_t[:L, hh * L:(hh + 1) * L],
                                             v_t[:L, bi, hh * 65:(hh + 1) * 65],
                                             start=True, stop=True)
                    if TPPIPE and len(pend) >= TPPIPE:
                        emit_tp(*pend.pop(0), ot_all)
                    r_t = rp.tile([128, 8], F32, tag="r")
                    o_t = op_.tile([128, D], BF16, tag="o")
                    if OUM:
                        nc.vector.reciprocal(r_t[:L, :8], sums[:L, :8])
                        _tt_eng(OENG[0]).tensor_tensor(
                            o_t[:L, :].rearrange("p (h x) -> p h x", x=64),
                            ou1[:L, :].rearrange("p (h x) -> p h x", x=64),
                            r_t[:L, :8].rearrange(
                                "p (h x) -> p h x", x=1).broadcast_to([L, 8, 64]),
                            op=OP.mult)
                    else:
                      for i, ou in enumerate((ouA, ouB)):
                        nc.vector.reciprocal(
                            r_t[:L, i * 4:(i + 1) * 4],
                            ou[:L, :].rearrange(
                                "p (h x) -> p h x", x=128)[:, :, 64:65])
                        _tt_eng(OENG[i]).tensor_tensor(
                            o_t[:L, i * 256:(i + 1) * 256].rearrange(
                                "p (h x) -> p h x", x=64),
                            ou[:L, :].rearrange(
                                "p (h x) -> p h x", x=128)[:, :, :64],
                            r_t[:L, i * 4:(i + 1) * 4].rearrange(
                                "p (h x) -> p h x", x=1).broadcast_to([L, 4, 64]),
                            op=OP.mult)
                    if TPPIPE:
                        pend.append((bi, o_t))
                    else:
                        emit_tp(bi, o_t, ot_all)
                while pend:
                    emit_tp(*pend.pop(0), ot_all)
                # Wo projection + residual
                for m in range(KT):
                    msl = slice(m * 128, (m + 1) * 128)
                    ps = pp.tile([128, 512], F32, tag="pp")
                    res_add = RES_DVE and R1ENG[m] != "a"
                    if not res_add:
                        # residual h loads first (start=True, full width);
                        # the Wo matmuls then accumulate onto it, safe with
                        # any column grouping
                        nc.tensor.matmul(ps[:, :TCH], ident_r[:],
                                         h_sb[m][:, gsl],
                                         start=True, stop=False)
                    if WOSPLIT:
                        # batch-outer: each column group runs its whole
                        # accumulation before the next start=True re-marks
                        # the bank's pending-zero flags (bank-granular)
                        for bi in range(CB):
                            bsl2 = slice(bi * L, (bi + 1) * L)
                            for k in range(KT):
                                nc.tensor.matmul(ps[:, bsl2], wo_l[:, k, msl],
                                                 ot_all[:, k, bsl2],
                                                 start=(k == 0) and res_add,
                                                 stop=(k == KT - 1) and
                                                      (res_add or bi == CB - 1))
                    else:
                        for k in range(KT):
                            nc.tensor.matmul(ps[:, :TCH], wo_l[:, k, msl],
                                             ot_all[:, k, :],
                                             start=(k == 0) and res_add,
                                             stop=(k == KT - 1))
                    if res_add:
                        _tt_eng(R1ENG[m]).tensor_tensor(
                            h_sb[m][:, gsl], ps[:, :TCH],
                            h_sb[m][:, gsl], op=OP.add)
                    else:
                        nc.scalar.copy(h_sb[m][:, gsl], ps[:, :TCH])
                if QKVPIPE:
                    while nxt_units:
                        nxt_units.pop(0)()
                    cur = nxt
                elif g + 1 < NT:
                    cur = alloc_qkv(g + 1)
                    for u in cur[3]:
                        u()
            def emit_out(t, split=1):
                tsl = slice(t * TCH, (t + 1) * TCH)
                ps = pp.tile([128, 512], F32, tag="pp")
                for k in range(KT):
                    nc.tensor.matmul(ps[:C, :TCH], wout_sb[:, k, :],
                                     h_sb[k][:, tsl],
                                     start=(k == 0), stop=(k == KT - 1))
                o_f = outp.tile([128, TCH], F32, tag="outc")
                w = TCH // split
                for s in range(split):
                    ssl = slice(s * w, (s + 1) * w)
                    dsl = slice(t * TCH + s * w, t * TCH + (s + 1) * w)
                    if os.environ.get("ANOM_OUTACT", "1") == "1":
                        nc.scalar.copy(o_f[:C, ssl], ps[:C, ssl])
                    else:
                        nc.vector.tensor_copy(o_f[:C, ssl], ps[:C, ssl])
                    nc.sync.dma_start(out=out_d[:, dsl], in_=o_f[:C, ssl])

            # FFN phase (trailing chunks; with FFNI all but the last were
            # emitted inside the attention loop). Between FFN pairs the PE
            # gets filler work: the next layer's chunk-0 q/k/v (LPIPE), or
            # on the last layer the output projection (OUTI).
            OUTI = os.environ.get("ANOM_OUTI", "1") == "1"
            last = lp_i == n_layer_passes - 1
            tail_ffn = last and os.environ.get("ANOM_TAILFFN", "1") == "1"
            TAILR2 = os.environ.get("ANOM_TAILR2", "adad")
            carry = None
            il_units = []  # (min_gp, unit): unit only legal once the FFN
            # pair before min_gp has been emitted (it reads h those wrote)
            if LPIPE and wt_next is not None:
                carry = alloc_qkv(0, wt_next[0])
                il_units = [(2, u) for u in carry[3]]
            g0 = NT - 1 if FFNI else 0
            if OUTI and last:
                il_units += [(t - t % 2 + 2, lambda t=t: emit_out(t))
                             for t in range(NT)]

            cur_gp = [0]

            def _fill():
                if il_units and il_units[0][0] <= cur_gp[0]:
                    il_units.pop(0)[1]()

            def _out_after(t):
                # the very last chunk streams out in pieces so the final
                # DMA chain overlaps its own copies
                if OUTI and last:
                    emit_out(t, split=2 if t == NT - 1 else 1)

            if FFNPAIR:
                for gp in range(g0, NT, 2):
                    cur_gp[0] = gp
                    if gp + 1 < NT:
                        emit_ffn_pair(gp, gp + 1, fill=_fill)
                    else:
                        emit_ffn(gp)
                        _out_after(gp)
                while il_units:
                    il_units.pop(0)[1]()
            elif FFNPIPE:
                ys = []
                for g in range(g0, NT):
                    ys.append((g, emit_ffn1(g)))
                    if len(ys) >= 2:
                        emit_ffn2(*ys.pop(0))
                while ys:
                    emit_ffn2(*ys.pop(0))
            else:
                for g in range(g0, NT):
                    emit_ffn(g)
                    _out_after(g)
                    for _ in range(2):
                        if il_units and il_units[0][0] <= g + 1:
                            il_units.pop(0)[1]()
                while il_units:
                    il_units.pop(0)[1]()
            if last and not OUTI:
                for t in range(NT):
                    emit_out(t)
            wt_cur = wt_next

    return _legalize_waits(nc)


def _bf(a):
    return np.ascontiguousarray(a).astype(ml_dtypes.bfloat16)


def _x(a):
    return _bf(a) if XBF else _r32(a)


def _f8(a):
    return np.ascontiguousarray(a).astype(ml_dtypes.float8_e4m3fn)


def _r32(a):
    """Round to the reduced-dtype grid (f32r: 10 explicit mantissa bits)."""
    if os.environ.get("ANOM_RDT", "f32r") == "bf16":
        return _bf(a)
    a = np.ascontiguousarray(a, np.float32)
    u = a.view(np.uint32).copy()
    u = (u + 0x1000) & 0xFFFFE000
    return u.view(np.float32)


# Without OUM, o features are written evens-first (heads 0,2,4,6 then
# 1,3,5,7) and Wo's input-feature rows are permuted to match; with OUM the
# o features are in natural head order.
_PERM_DIN = (np.arange(D) if OUM else
             np.concatenate([np.arange(h * DK, (h + 1) * DK)
                             for h in (0, 2, 4, 6, 1, 3, 5, 7)]))


def prep_weights(tok_w, pe, Wq, Wk, Wv, Wo, W1, W2, proj_w):
    """Host-side weight reorganization (shared across cores)."""
    scale = 1.0 / math.sqrt(DK)
    # conv unfold: W_unf[55d + c, o] = tok_w[o, c, d]
    wemb = np.ascontiguousarray(np.transpose(tok_w, (2, 1, 0))).reshape(C3, D)
    # projection weights as lhsT tiles: w[p, l, k, j] = W[l, j, 128k + p]
    def proj_lhsT(W):  # [NL, D_out, D_in] -> [128, NL, KT, D_out]
        return np.ascontiguousarray(
            np.transpose(W, (2, 0, 1)).reshape(KT, 128, NL, W.shape[1])
            .transpose(1, 2, 0, 3))
    eye = np.eye(128, dtype=np.float32)
    m = {
        "identb": _bf(eye), "identr": _r32(eye),
        "wemb0": _x(wemb[:128]), "wemb1": _x(wemb[128:]),
        "pe": np.ascontiguousarray(
            np.ascontiguousarray(pe.T).reshape(KT, 128, L).transpose(1, 0, 2)),
        # wqkv[p, l, i, k, j]: i = 0/1/2 for scaled-Q/K/V lhsT tiles
        "wqkv": _r32(np.stack([proj_lhsT(Wq * scale), proj_lhsT(Wk),
                               proj_lhsT(Wv)], axis=2)),
        "wo": (_f8(proj_lhsT(Wo[:, :, _PERM_DIN])).view(np.uint8) if WODR
               else _bf(proj_lhsT(Wo[:, :, _PERM_DIN]))),
        "w1": _r32(proj_lhsT(W1)),
        # w2[p, l, m, j] = W2[l, 128m + j, p]   (p over DFF=64)
        "w2": _r32(np.transpose(W2, (2, 0, 1)).reshape(DFF, NL, KT, 128)),
        # wout[p, k, j] = proj_w[j, 128k + p]
        "wout": _r32(np.ascontiguousarray(proj_w.T).reshape(KT, 128, C)
                     .transpose(1, 0, 2)),
    }
    return m


def prep_xcat(xs):
    """Per-core input: xs [BL, L, C] -> circular-unfolded feature-major
    [165, BL*L], split into [128, .] + [37, .]."""
    xt = np.ascontiguousarray(np.transpose(xs, (2, 0, 1)))    # [C, BL, L]
    rows = [np.roll(xt, 1 - d, axis=2) for d in range(3)]     # x[t+d-1]
    xcat = np.concatenate(rows, axis=0).reshape(C3, TOK)
    return _x(xcat[:128]), _x(xcat[128:])


_NC_CACHE = {}


def get_nc():
    if "nc" not in _NC_CACHE:
        _NC_CACHE["nc"] = build_nc()
    return _NC_CACHE["nc"]


def make_in_maps(inputs):
    x = np.asarray(inputs["x"], np.float32)
    wm = prep_weights(np.asarray(inputs["tok_w"], np.float32),
                      np.asarray(inputs["pe"], np.float32),
                      np.asarray(inputs["Wq"], np.float32),
                      np.asarray(inputs["Wk"], np.float32),
                      np.asarray(inputs["Wv"], np.float32),
                      np.asarray(inputs["Wo"], np.float32),
                      np.asarray(inputs["W1"], np.float32),
                      np.asarray(inputs["W2"], np.float32),
                      np.asarray(inputs["proj_w"], np.float32))
    in_maps = []
    for c in range(NCORES):
        x0, x1 = prep_xcat(x[c * BL:(c + 1) * BL])
        in_maps.append({**wm, "xcat0": x0, "xcat1": x1})
    return in_maps


def assemble_out(results):
    # per-core out [C, TOK] feature-major -> [B, L, C]
    outs = [np.asarray(r["out"], np.float32).reshape(C, BL, L).transpose(1, 2, 0)
            for r in results]
    return np.concatenate(outs, axis=0)


def kernel(**inputs) -> np.ndarray:
    nc = get_nc()
    in_maps = make_in_maps(inputs)
    res = run_bass_kernel_spmd(nc, in_maps, core_ids=list(range(NCORES)))
    return assemble_out(res.results)

